# revision 12
# baseline (speedup 1.0000x reference)
"""GRU kernel for 8 trn2 NeuronCores.

Sharding: data-parallel over batch (B=128 -> 16 per core), weights replicated.
All heavy math in bf16 on the TensorEngine; fp32 PSUM accumulation.

Per-core structure:
  Phase 1: XP[t] = [x_t@W_xr + b_r | x_t@W_xz + b_z | x_t@W_xh + b_h]  (time-parallel)
  Phase 2: 512 sequential GRU steps. Recurrent matmuls keep h^T strips as the
           stationary operand (M=16 batch cols) and stream the weights; the
           precomputed XP terms enter the same PSUM accumulation via an
           identity-stationary matmul (avoids slow 16-partition DVE adds).
           h^T / (R*h)^T strips are produced with PE transposes.
  Phase 3: Y[t] = h_t@W_hq + b_q from the h^T history      (time-parallel)
"""
import sys
sys.path.insert(0, "/opt/trn_rl_repo")

import numpy as np
import ml_dtypes

import concourse.bass as bass
import concourse.mybir as mybir
from concourse.bass_utils import run_bass_kernel_spmd

AF = mybir.ActivationFunctionType
BF16 = mybir.dt.bfloat16
F32 = mybir.dt.float32

N_CORES = 8
B = 128
BL = B // N_CORES          # 16 batch rows per core
HD = 1024
KS = HD // 128             # 8 contraction strips
NCHUNK = 512               # matmul moving free-dim chunk


def build(T):
    """Build the per-core Bass program for T timesteps (tokens = T*BL)."""
    TOK = T * BL
    NT = TOK // 128        # 128-token tiles for phases 1/3
    assert NT * 128 == TOK

    nc = bass.Bass()

    xT = nc.declare_dram_parameter("xT", [HD, TOK], BF16, isOutput=False)
    h0b = nc.declare_dram_parameter("h0b", [BL, HD], BF16, isOutput=False)
    h0T = nc.declare_dram_parameter("h0T", [128, 128], BF16, isOutput=False)
    wzr = nc.declare_dram_parameter("wzr", [128, KS * 2048], BF16, isOutput=False)
    whh = nc.declare_dram_parameter("whh", [128, KS * HD], BF16, isOutput=False)
    whq = nc.declare_dram_parameter("whq", [128, KS * HD], BF16, isOutput=False)
    wx3 = nc.declare_dram_parameter("wx3", [128, KS * 3072], BF16, isOutput=False)
    b3 = nc.declare_dram_parameter("b3", [1, 3072], BF16, isOutput=False)
    bq = nc.declare_dram_parameter("bq", [1, HD], BF16, isOutput=False)
    ones = nc.declare_dram_parameter("ones", [1, 128], BF16, isOutput=False)
    ident = nc.declare_dram_parameter("ident", [128, 128], BF16, isOutput=False)

    y_out = nc.declare_dram_parameter("y", [TOK, HD], F32, isOutput=True)
    hf_out = nc.declare_dram_parameter("hf", [BL, HD], F32, isOutput=True)

    xp_d = nc.dram_tensor("xp_d", [TOK, 3072], BF16)
    hist = nc.dram_tensor("hist", [T, 128, 128], BF16)

    from contextlib import ExitStack
    with ExitStack() as _es:
        _n = [0]
        def _sb(shape, dt):
            _n[0] += 1
            return _es.enter_context(nc.sbuf_tensor(f"sb{_n[0]}", shape, dt))
        def _sem(name):
            return _es.enter_context(nc.semaphore(name))

        # ---- persistent SBUF ----
        wx3_s = _sb([128, KS * 3072], BF16)
        wzr_s = _sb([128, KS * 2048], BF16)
        whh_s = _sb([128, KS * HD], BF16)
        whq_s = _sb([128, KS * HD], BF16)
        xt_s = _sb([128, 2, 8, 128], BF16)      # phase1 lhsT tiles
        p1_st = _sb([128, 2, 3072], BF16)       # phase1 drain stage
        xp_s = _sb([16, 2, 3072], BF16)         # per-step XP
        h_s = _sb([16, HD], BF16)               # h (normal layout)
        r_s = _sb([16, HD], BF16)
        z_s = _sb([16, HD], BF16)
        ht_s = _sb([16, HD], BF16)              # tanh output
        tmp_s = _sb([16, HD], BF16)
        v_s = _sb([16, HD], BF16)               # R*h
        hT_s = _sb([128, 128], BF16)            # h^T strips
        vT_s = _sb([128, 128], BF16)            # (R*h)^T strips
        id_s = _sb([128, 128], BF16)
        ones_s = _sb([1, 128], BF16)
        b3_s = _sb([1, 3072], BF16)
        bq_s = _sb([1, HD], BF16)
        ht3_s = _sb([128, 2, 8, 128], BF16)     # phase3 lhsT tiles
        y_st = _sb([128, 2, HD], F32)           # phase3 drain stage
        hf_st = _sb([16, HD], F32)
        # ---- semaphores ----
        s_w = _sem("s_w"); s_xt = _sem("s_xt"); s_xp = _sem("s_xp")
        s_h3 = _sem("s_h3"); s_hist = _sem("s_hist"); s_xpo = _sem("s_xpo")
        s_yo = _sem("s_yo")
        p_p1 = _sem("p_p1"); p_r = _sem("p_r"); p_z = _sem("p_z")
        p_vt = _sem("p_vt"); p_h = _sem("p_h"); p_ht = _sem("p_ht")
        p_y = _sem("p_y")
        a_r = _sem("a_r"); a_z = _sem("a_z"); a_t = _sem("a_t")
        a_vtsb = _sem("a_vtsb"); a_ht = _sem("a_ht")
        c_d0 = _sem("c_d0"); c_d1 = _sem("c_d1")
        c_ya = _sem("c_ya"); c_yd = _sem("c_yd")
        d_v = _sem("d_v"); d_hn = _sem("d_hn")

        NW = 11  # const-load DMA count
        # ======================= PHASE 1 =======================
        with (
            nc.psum_tensor([128, 1536], F32) as ps1a,
            nc.psum_tensor([128, 1536], F32) as ps1b,
            nc.Block() as blk,
        ):
            @blk.sync
            def _(e):
                for wi, (dst, src) in enumerate((
                    (wx3_s[:, :KS * 1536], wx3[:, :KS * 1536]),
                    (wx3_s[:, KS * 1536:], wx3[:, KS * 1536:]),
                    (wzr_s[:], wzr[:]), (whh_s[:], whh[:]),
                    (whq_s[:], whq[:]), (id_s[:], ident[:]), (ones_s[:], ones[:]),
                    (b3_s[:], b3[:]), (bq_s[:], bq[:]), (hT_s[:], h0T[:]),
                    (h_s[:], h0b[:]),
                )):
                    if wi >= 1:
                        e.wait_ge(s_w, 16 * wi)
                    e.dma_start(out=dst, in_=src).then_inc(s_w, 16)
                # memset-equivalent: hf_st unused until end
                xTr = xT.rearrange("(k p) n -> p k n", p=128)
                for tt in range(NT):
                    if tt >= 2:
                        e.wait_ge(p_p1, tt - 1)
                    if tt >= 1:
                        e.wait_ge(s_xt, 16 * tt)
                    e.dma_start(
                        out=xt_s[:, tt % 2], in_=xTr[:, :, tt * 128:(tt + 1) * 128]
                    ).then_inc(s_xt, 16)

            @blk.tensor
            def _(e):
                e.wait_ge(s_w, 16 * NW)
                for tt in range(NT):
                    par = tt % 2
                    e.wait_ge(s_xt, 16 * (tt + 1))
                    for half, ps in ((0, ps1a), (1, ps1b)):
                        if tt >= 1:
                            e.wait_ge(c_d0 if half == 0 else c_d1, tt)
                        for k in range(KS + 1):
                            lhsT = ones_s[:] if k == KS else xt_s[:, par, k, :]
                            for c in range(3):
                                off = half * 1536 + c * 512
                                rhs = (b3_s[0:1, off:off + 512] if k == KS
                                       else wx3_s[:, k * 3072 + off: k * 3072 + off + 512])
                                mm = e.matmul(ps[:, c * 512:(c + 1) * 512], lhsT, rhs,
                                              start=(k == 0), stop=(k == KS))
                        if half == 1:
                            mm.then_inc(p_p1, 1)

            @blk.scalar
            def _(e):
                for tt in range(NT):
                    par = tt % 2
                    e.wait_ge(p_p1, tt + 1)
                    if tt >= 2:
                        e.wait_ge(s_xpo, 16 * (tt - 1))
                    e.activation(p1_st[:, par, 0:1536], ps1a[:], AF.Copy).then_inc(c_d0, 1)

            @blk.vector
            def _(e):
                for tt in range(NT):
                    par = tt % 2
                    e.wait_ge(p_p1, tt + 1)
                    if tt >= 2:
                        e.wait_ge(s_xpo, 16 * (tt - 1))
                    e.tensor_copy(p1_st[:, par, 1536:3072], ps1b[:]).then_inc(c_d1, 1)

            @blk.gpsimd
            def _(e):
                for tt in range(NT):
                    par = tt % 2
                    e.wait_ge(c_d0, tt + 1)
                    e.wait_ge(c_d1, tt + 1)
                    if tt >= 1:
                        e.wait_ge(s_xpo, 16 * tt)
                    e.dma_start(
                        out=xp_d[tt * 128:(tt + 1) * 128, :], in_=p1_st[:, par, :]
                    ).then_inc(s_xpo, 16)

        # ======================= PHASE 2: recurrence =======================
        with (
            nc.psum_tensor([16, HD], F32) as ps_r,
            nc.psum_tensor([16, HD], F32) as ps_z,
            nc.psum_tensor([16, HD], F32) as ps_h,
            nc.psum_tensor([128, 128], BF16) as ps_t,
            nc.psum_tensor([128, 128], BF16) as ps_t2,
            nc.Block() as blk,
        ):
            id16 = id_s[0:16, 0:16]

            @blk.sync
            def _(e):
                e.wait_ge(s_xpo, 16 * NT)
                for t in range(T):
                    if t >= 2:
                        e.wait_ge(p_h, t - 1)
                    if t >= 1:
                        e.wait_ge(s_xp, 16 * t)
                    e.dma_start(
                        out=xp_s[:, t % 2, :],
                        in_=xp_d[t * BL:(t + 1) * BL, :],
                    ).then_inc(s_xp, 16)

            @blk.tensor
            def _(e):
                for t in range(T):
                    par = t % 2
                    xpp = xp_s[:, par]
                    # ---- R = xp_r + h @ W_hr ----
                    e.wait_ge(s_xp, 16 * (t + 1))
                    e.wait_ge(a_ht, t)
                    e.wait_ge(a_r, t)
                    for c in range(2):
                        cs = slice(c * 512, (c + 1) * 512)
                        e.matmul(ps_r[:, cs], id16, xpp[:, cs], start=True, stop=False)
                        for k in range(KS):
                            mm = e.matmul(ps_r[:, cs], hT_s[:, k * 16:(k + 1) * 16],
                                     wzr_s[:, k * 2048 + c * 512: k * 2048 + c * 512 + 512],
                                     start=False, stop=(k == KS - 1))
                    mm.then_inc(p_r, 1)
                    # ---- Z = xp_z + h @ W_hz ----
                    e.wait_ge(a_z, t)
                    for c in range(2):
                        cs = slice(c * 512, (c + 1) * 512)
                        e.matmul(ps_z[:, cs], id16, xpp[:, 1024 + c * 512: 1024 + c * 512 + 512],
                                 start=True, stop=False)
                        for k in range(KS):
                            mm = e.matmul(ps_z[:, cs], hT_s[:, k * 16:(k + 1) * 16],
                                     wzr_s[:, k * 2048 + 1024 + c * 512: k * 2048 + 1024 + c * 512 + 512],
                                     start=False, stop=(k == KS - 1))
                    mm.then_inc(p_z, 1)
                    # ---- vT = (R*h)^T strips ----
                    e.wait_ge(d_v, t + 1)
                    e.wait_ge(a_vtsb, t)
                    for k in range(KS):
                        mm = e.transpose(ps_t[:, k * 16:(k + 1) * 16],
                                    v_s[:, k * 128:(k + 1) * 128], id16)
                    mm.then_inc(p_vt, 1)
                    # ---- Htilda_pre = xp_h + (R*h) @ W_hh ----
                    e.wait_ge(a_vtsb, t + 1)
                    e.wait_ge(a_t, t)
                    for c in range(2):
                        cs = slice(c * 512, (c + 1) * 512)
                        e.matmul(ps_h[:, cs], id16, xpp[:, 2048 + c * 512: 2048 + c * 512 + 512],
                                 start=True, stop=False)
                        for k in range(KS):
                            mm = e.matmul(ps_h[:, cs], vT_s[:, k * 16:(k + 1) * 16],
                                     whh_s[:, k * 1024 + c * 512: k * 1024 + c * 512 + 512],
                                     start=False, stop=(k == KS - 1))
                    mm.then_inc(p_h, 1)
                    # ---- h_new^T strips ----
                    e.wait_ge(d_hn, t + 1)
                    e.wait_ge(a_ht, t)
                    for k in range(KS):
                        mm = e.transpose(ps_t2[:, k * 16:(k + 1) * 16],
                                    h_s[:, k * 128:(k + 1) * 128], id16)
                    mm.then_inc(p_ht, 1)

            @blk.scalar
            def _(e):
                for t in range(T):
                    e.wait_ge(p_r, t + 1)
                    e.wait_ge(d_v, t)
                    e.activation(r_s[:], ps_r[:], AF.Sigmoid).then_inc(a_r, 1)
                    e.wait_ge(p_vt, t + 1)
                    e.wait_ge(p_h, t)
                    e.activation(vT_s[:], ps_t[:], AF.Copy).then_inc(a_vtsb, 1)
                    e.wait_ge(p_z, t + 1)
                    e.wait_ge(d_hn, t)
                    e.activation(z_s[:], ps_z[:], AF.Sigmoid).then_inc(a_z, 1)
                    e.wait_ge(p_h, t + 1)
                    e.activation(ht_s[:], ps_h[:], AF.Tanh).then_inc(a_t, 1)
                    e.wait_ge(p_ht, t + 1)
                    e.wait_ge(s_hist, 16 * t)
                    e.activation(hT_s[:], ps_t2[:], AF.Copy).then_inc(a_ht, 1)
                # final h -> fp32 stage
                e.wait_ge(d_hn, T)
                e.activation(hf_st[:], h_s[:], AF.Copy).then_inc(a_ht, 1)

            @blk.vector
            def _(e):
                e.wait_ge(s_w, 16 * NW)
                for t in range(T):
                    e.wait_ge(a_r, t + 1)
                    e.wait_ge(p_vt, t)
                    e.tensor_mul(v_s[:], r_s[:], h_s[:]).then_inc(d_v, 1)
                    e.wait_ge(a_t, t + 1)
                    e.wait_ge(a_z, t + 1)
                    e.wait_ge(p_ht, t)
                    e.tensor_sub(tmp_s[:], h_s[:], ht_s[:])
                    e.drain()
                    e.tensor_mul(tmp_s[:], tmp_s[:], z_s[:])
                    e.drain()
                    e.tensor_add(h_s[:], ht_s[:], tmp_s[:]).then_inc(d_hn, 1)
                    e.drain()

            @blk.gpsimd
            def _(e):
                for t in range(T):
                    e.wait_ge(a_ht, t + 1)
                    if t >= 1:
                        e.wait_ge(s_hist, 16 * t)
                    e.dma_start(out=hist[t], in_=hT_s[:]).then_inc(s_hist, 16)
                # hf out (a_ht reaches T+1 after hf_st copy)
                e.wait_ge(a_ht, T + 1)
                e.wait_ge(s_hist, 16 * T)
                e.dma_start(out=hf_out[:], in_=hf_st[:]).then_inc(s_hist, 16)

        # ======================= PHASE 3: y = h @ W_hq + b_q =======================
        with (
            nc.psum_tensor([128, 2 * HD], F32) as ps_y,
            nc.Block() as blk,
        ):
            histr = hist.rearrange("t p b -> p t b")

            @blk.sync
            def _(e):
                for tt in range(NT):
                    e.wait_ge(s_hist, 16 * (tt + 1) * 8)
                    if tt >= 2:
                        e.wait_ge(p_y, tt - 1)
                    for k in range(KS):
                        if tt * KS + k >= 1:
                            e.wait_ge(s_h3, 16 * (tt * KS + k))
                        e.dma_start(
                            out=ht3_s[:, tt % 2, k, :],
                            in_=histr[:, tt * 8:(tt + 1) * 8, k * 16:(k + 1) * 16],
                        ).then_inc(s_h3, 16)

            @blk.tensor
            def _(e):
                for tt in range(NT):
                    par = tt % 2
                    e.wait_ge(s_h3, 16 * KS * (tt + 1))
                    if tt >= 2:
                        e.wait_ge(c_ya, tt - 1)
                        e.wait_ge(c_yd, tt - 1)
                    for c in range(2):
                        cs = slice(par * HD + c * 512, par * HD + (c + 1) * 512)
                        e.matmul(ps_y[:, cs], ones_s[:], bq_s[0:1, c * 512:(c + 1) * 512],
                                 start=True, stop=False)
                        for k in range(KS):
                            lhsT = ht3_s[:, par, k, :]
                            mm = e.matmul(ps_y[:, cs], lhsT,
                                     whq_s[:, k * 1024 + c * 512: k * 1024 + c * 512 + 512],
                                     start=False, stop=(k == KS - 1))
                    mm.then_inc(p_y, 1)

            @blk.scalar
            def _(e):
                for tt in range(NT):
                    par = tt % 2
                    e.wait_ge(p_y, tt + 1)
                    if tt >= 2:
                        e.wait_ge(s_yo, 16 * (tt - 1))
                    e.activation(y_st[:, par, 0:512], ps_y[:, par * HD: par * HD + 512],
                                 AF.Copy).then_inc(c_ya, 1)

            @blk.vector
            def _(e):
                for tt in range(NT):
                    par = tt % 2
                    e.wait_ge(p_y, tt + 1)
                    if tt >= 2:
                        e.wait_ge(s_yo, 16 * (tt - 1))
                    e.tensor_copy(y_st[:, par, 512:1024],
                                  ps_y[:, par * HD + 512: par * HD + 1024]).then_inc(c_yd, 1)

            @blk.gpsimd
            def _(e):
                for tt in range(NT):
                    par = tt % 2
                    e.wait_ge(c_ya, tt + 1)
                    e.wait_ge(c_yd, tt + 1)
                    if tt >= 1:
                        e.wait_ge(s_yo, 16 * tt)
                    e.dma_start(out=y_out[tt * 128:(tt + 1) * 128, :],
                                in_=y_st[:, par, :]).then_inc(s_yo, 16)

    return nc


def _to_bf16(a):
    return np.asarray(a, dtype=ml_dtypes.bfloat16)


def _strips(w, cols):
    """[1024, cols] -> [128, 8*cols] strip-major for SBUF rhs tiles."""
    return np.ascontiguousarray(
        w.reshape(KS, 128, cols).transpose(1, 0, 2).reshape(128, KS * cols)
    )


_BUILD_CACHE = {}


def kernel(inputs, H, W_xz, W_hz, b_z, W_xr, W_hr, b_r, W_xh, W_hh, b_h, W_hq, b_q):
    T = inputs.shape[0]
    inputs = np.asarray(inputs, dtype=np.float32)
    H = np.asarray(H, dtype=np.float32)

    wzr_full = np.concatenate([np.asarray(W_hr), np.asarray(W_hz)], axis=1)
    wx3_full = np.concatenate([np.asarray(W_xr), np.asarray(W_xz), np.asarray(W_xh)], axis=1)
    b3_full = np.concatenate([np.asarray(b_r), np.asarray(b_z), np.asarray(b_h)])

    wzr_h = _to_bf16(_strips(wzr_full, 2048))
    whh_h = _to_bf16(_strips(np.asarray(W_hh), HD))
    whq_h = _to_bf16(_strips(np.asarray(W_hq), HD))
    wx3_h = _to_bf16(_strips(wx3_full, 3072))
    b3_h = _to_bf16(b3_full.reshape(1, 3072))
    bq_h = _to_bf16(np.asarray(b_q).reshape(1, HD))
    ones_h = np.ones((1, 128), dtype=ml_dtypes.bfloat16)
    id_h = _to_bf16(np.eye(128, dtype=np.float32))

    in_maps = []
    for c in range(N_CORES):
        sl = slice(c * BL, (c + 1) * BL)
        Xc = inputs[:, sl, :]                               # [T, 16, 1024]
        xT_h = _to_bf16(np.ascontiguousarray(Xc.reshape(T * BL, HD).T))
        Hc = H[sl, :]                                       # [16, 1024]
        h0T_h = _to_bf16(
            np.ascontiguousarray(
                Hc.T.reshape(KS, 128, BL).transpose(1, 0, 2).reshape(128, 128)
            )
        )
        in_maps.append({
            "xT": xT_h, "h0b": _to_bf16(Hc), "h0T": h0T_h,
            "wzr": wzr_h, "whh": whh_h, "whq": whq_h, "wx3": wx3_h,
            "b3": b3_h, "bq": bq_h, "ones": ones_h, "ident": id_h,
        })

    if T not in _BUILD_CACHE:
        _BUILD_CACHE[T] = build(T)
    nc = _BUILD_CACHE[T]

    res = run_bass_kernel_spmd(nc, in_maps, list(range(N_CORES)))

    Ys = np.empty((T, B, HD), dtype=np.float32)
    Hf = np.empty((B, HD), dtype=np.float32)
    for c in range(N_CORES):
        sl = slice(c * BL, (c + 1) * BL)
        Ys[:, sl, :] = res.results[c]["y"].reshape(T, BL, HD)
        Hf[sl, :] = res.results[c]["hf"]
    return Ys, Hf
